# revision 13
# baseline (speedup 1.0000x reference)
"""RBF-kernel attention on 8 TRN2 NeuronCores.

Math (per reference): scores = exp(-gamma*SCALE*dist), dist = ||qh_s - kh_t||^2,
kept only on the STRICT upper triangle (t > s), out = scores @ vh, then @ Wo.

Factorization: -c*dist = 2c*qk[s,t] - c*kn[t] - c*qn[s], c = gamma_h*SCALE.
The 2c factor is folded into Wk HOST-SIDE. The -c*qn[s] term rides along the
SAME qk matmul as an augmented contraction row (matmul cost depends only on
output size). The -c*kn[t] term enters as the PER-PARTITION bias of the Exp
activation (partition dim of the score tile is t). One Exp per head-half
yields the TRUE scores in (0,1]; v needs no pre/post scaling.

Per-head k/q tiles, partition-aligned with the projection PSUM layout (every
compute op keeps in/out partition ranges identical):
  even local heads (0,2): data rows [0:64],   aug row 64  -> qk slice [0:65]
  odd  local heads (1,3): data rows [64:128], aug row 32, zero rows [33:64]
                                              -> qk slice [32:128]
khT aug row = ones (const DMA), qhT aug row = -c*qn (computed; the qn matmul
uses a replicated-column lhsT so every PSUM partition holds qn, making the
single-row copy partition-aligned).

Strict-upper masking: diagonal-crossing t-tiles shrink to their column span;
only the last 128 columns are partial and get a [128,128] strict-lower mask
multiply on DVE. max(dist,0) is a no-op off the masked diagonal.

Sharding: core c = (batch b=c//4, head-group g=c%4); each core computes 4
heads of one batch and a PARTIAL final output [S, E] (bf16) through its Wo
row slice; the host sums the 4 partials per batch. No collectives.
"""
import sys
sys.path.insert(0, '/opt/trn_rl_repo')
import math
import numpy as np
import ml_dtypes

from concourse import bass, bacc, tile, mybir, bass_utils

F32 = mybir.dt.float32
BF16 = mybir.dt.bfloat16
AF = mybir.ActivationFunctionType
ALU = mybir.AluOpType

B, S, E, H = 2, 2048, 1024, 16
D = E // H
SCALE = 1.0 / math.sqrt(D)
N_CORES = 8
HPC = H // 4
HD = HPC * D            # 256
NKT = E // 128          # 8
NST = S // 128          # 16
NSC = S // 512          # 4

# per local head: (tile partition range for data, aug row, qk slice lo/hi)
HEAD_DATA = [(0, 64), (64, 128), (0, 64), (64, 128)]
HEAD_AUG = [64, 32, 64, 32]
HEAD_SLICE = [(0, 128), (0, 128), (0, 128), (0, 128)]

_nc_cache = {}


def build_graph():
    if 'nc' in _nc_cache:
        return _nc_cache['nc']
    nc = bacc.Bacc("TRN2", target_bir_lowering=False, debug=False,
                   num_devices=N_CORES)

    qT_in = nc.dram_tensor("qbT", [E, S], BF16, kind="ExternalInput").ap()
    wq_in = nc.dram_tensor("wq", [E, HD], BF16, kind="ExternalInput").ap()
    wk_in = nc.dram_tensor("wk", [E, HD], BF16, kind="ExternalInput").ap()
    wv_in = nc.dram_tensor("wv", [E, HD], BF16, kind="ExternalInput").ap()
    wo_in = nc.dram_tensor("wo", [HD, E], BF16, kind="ExternalInput").ap()
    hk_in = nc.dram_tensor("hselK", [128, HPC], BF16, kind="ExternalInput").ap()
    # hselQ128[:, 128*i + p] = -c_i on head i's data rows, for all p
    hq_in = nc.dram_tensor("hselQ128", [128, 128 * HPC], BF16,
                           kind="ExternalInput").ap()
    out_d = nc.dram_tensor("out", [S, E], BF16, kind="ExternalOutput").ap()

    ones_c = nc.inline_tensor(np.ones((1, S), dtype=ml_dtypes.bfloat16),
                              name="ones_row")
    zeros_c = nc.inline_tensor(np.zeros((64, S), dtype=ml_dtypes.bfloat16),
                               name="zeros_blk")
    # strict-lower [128,128]: keep iff p > f (t > s inside the diagonal band)
    pp = np.arange(128)[:, None]
    ff = np.arange(128)[None, :]
    mask_np = (pp > ff).astype(ml_dtypes.bfloat16)
    mask_c = nc.inline_tensor(mask_np, name="masktri")

    with tile.TileContext(nc) as tc:
        with tc.tile_pool(name="persist", bufs=1) as P, \
             tc.tile_pool(name="wpool", bufs=1) as WP:
            qT = [P.tile([128, S], BF16, name=f"qT{e}", tag=f"qT{e}")
                  for e in range(NKT)]
            khT = [P.tile([128, S], BF16, name=f"khT{h}", tag=f"khT{h}")
                   for h in range(HPC)]
            qhT = [P.tile([128, S], BF16, name=f"qhT{h}", tag=f"qhT{h}")
                   for h in range(HPC)]
            vp = [P.tile([128, HD], BF16, name=f"vp{w}", tag=f"vp{w}")
                  for w in range(NST)]
            outT = [P.tile([128, S], BF16, name=f"outT{m}", tag=f"outT{m}")
                    for m in range(2)]
            # knT[:, 4*ti + h] = -c_h * kn_h[128*ti + p]  (f32, exp bias)
            knT = P.tile([128, 4 * NST], F32, name="knT", tag="knT")
            hselK_t = P.tile([128, HPC], BF16, name="hselK", tag="hselK")
            hselQ_t = P.tile([128, 128 * HPC], BF16, name="hselQ128",
                             tag="hselQ128")
            mask_t = P.tile([128, 128], BF16, name="masktri", tag="masktri")
            wqb = [WP.tile([128, HD], BF16, name=f"wqb{k}", tag=f"wqb{k}")
                   for k in range(NKT)]
            wkb = [WP.tile([128, HD], BF16, name=f"wkb{k}", tag=f"wkb{k}")
                   for k in range(NKT)]
            wvb = [WP.tile([128, HD], BF16, name=f"wvb{k}", tag=f"wvb{k}")
                   for k in range(NKT)]
            wob = [WP.tile([128, E], BF16, name=f"wob{k}", tag=f"wob{k}")
                   for k in range(2)]

            # ---- DMA issue order: sync carries qT (+aug const rows),
            # scalar carries weights/consts, interleaved so the k-th khT
            # accumulation chain can start as soon as (qT[k], wk[k]) land.
            for k in range(NKT):
                nc.sync.dma_start(qT[k][:], qT_in[128 * k:128 * k + 128, :])
                nc.scalar.dma_start(wkb[k][:], wk_in[128 * k:128 * k + 128, :])
            for h in range(HPC):
                a = HEAD_AUG[h]
                # zero the non-data half (incl. aug row), then overwrite the
                # aug row: khT gets ones; qhT's is rewritten per chunk (-c*qn)
                zlo = 64 if a == 64 else 0
                nc.sync.dma_start(khT[h][zlo:zlo + 64, :], zeros_c.ap())
                nc.sync.dma_start(qhT[h][zlo:zlo + 64, :], zeros_c.ap())
                nc.sync.dma_start(khT[h][a:a + 1, :], ones_c.ap())
            nc.scalar.dma_start(hselK_t[:], hk_in)
            nc.scalar.dma_start(hselQ_t[:], hq_in)
            for k in range(NKT):
                nc.scalar.dma_start(wqb[k][:], wq_in[128 * k:128 * k + 128, :])
            nc.scalar.dma_start(mask_t[:], mask_c.ap())
            for k in range(NKT):
                nc.scalar.dma_start(wvb[k][:], wv_in[128 * k:128 * k + 128, :])
            for k in range(2):
                nc.scalar.dma_start(wob[k][:], wo_in[128 * k:128 * k + 128, :])

            # ---- khT projection: 8 concurrent PSUM chains (one per (m,n)),
            # consuming qT[k] in DMA arrival order. PSUM partition layout is
            # [head 2m | head 2m+1]; copies keep partition ranges aligned.
            with tc.tile_pool(name="ph", bufs=1, space="PSUM") as PH:
                ph = [PH.tile([128, 512], F32, name=f"ph{i}", tag=f"ph{i}")
                      for i in range(8)]
                for k in range(NKT):
                    for m in range(2):
                        for n in range(NSC):
                            nc.tensor.matmul(
                                ph[4 * m + n][:],
                                wkb[k][:, 128 * m:128 * m + 128],
                                qT[k][:, 512 * n:512 * n + 512],
                                start=(k == 0), stop=(k == NKT - 1))
                for m in range(2):
                    for n in range(NSC):
                        for hl in range(2):
                            lo, hi = HEAD_DATA[2 * m + hl]
                            nc.vector.tensor_copy(
                                khT[2 * m + hl][lo:hi, 512 * n:512 * n + 512],
                                ph[4 * m + n][64 * hl:64 * hl + 64, :])

            with tc.tile_pool(name="ps", bufs=6, space="PSUM") as PS, \
                 tc.tile_pool(name="otps", bufs=2, space="PSUM") as OT, \
                 tc.tile_pool(name="sq", bufs=3) as SQ, \
                 tc.tile_pool(name="ep", bufs=12) as EP:

                # kn bias columns: -c*kn[t] into knT[:, 4*ti + h]  (t-major)
                for m in range(2):
                    for n in range(NSC):
                        sq = SQ.tile([128, 512], BF16, name="sqk", tag="sq")
                        for hl in range(2):
                            lo, hi = HEAD_DATA[2 * m + hl]
                            src = khT[2 * m + hl][lo:hi,
                                                  512 * n:512 * n + 512]
                            nc.vector.tensor_tensor(
                                sq[lo:hi, :], src, src, op=ALU.mult)
                        ps = PS.tile([128, 512], F32, name="psk", tag="ps")
                        for r in range(4):
                            nc.tensor.matmul(
                                ps[:, 2 * r:2 * r + 2],
                                sq[:, 128 * r:128 * r + 128],
                                hselK_t[:, 2 * m:2 * m + 2],
                                start=True, stop=True)
                        for r in range(4):
                            ti = 4 * n + r
                            nc.vector.tensor_copy(
                                knT[:, 4 * ti + 2 * m:4 * ti + 2 * m + 2],
                                ps[:, 2 * r:2 * r + 2])

                def preamble(sj):
                    # qhT chunk sj + qn aug row (-c*qn, partition-aligned
                    # copy out of a replicated-column qn matmul)
                    for m in range(2):
                        ps = PS.tile([128, 512], F32, name="psq", tag="ps")
                        for k in range(NKT):
                            nc.tensor.matmul(
                                ps[:, 0:512], wqb[k][:, 128 * m:128 * m + 128],
                                qT[k][:, 512 * sj:512 * sj + 512],
                                start=(k == 0), stop=(k == NKT - 1))
                        for hl in range(2):
                            lo, hi = HEAD_DATA[2 * m + hl]
                            nc.vector.tensor_copy(
                                qhT[2 * m + hl][lo:hi, 512 * sj:512 * sj + 512],
                                ps[64 * hl:64 * hl + 64, 0:512])
                        sq = SQ.tile([128, 512], BF16, name="sqq", tag="sq")
                        for hl in range(2):
                            lo, hi = HEAD_DATA[2 * m + hl]
                            src = qhT[2 * m + hl][lo:hi,
                                                  512 * sj:512 * sj + 512]
                            nc.vector.tensor_tensor(
                                sq[lo:hi, :], src, src, op=ALU.mult)
                        for j in range(2):
                            h = 2 * m + j
                            a = HEAD_AUG[h]
                            ps2 = PS.tile([128, 512], F32, name="psn",
                                          tag="ps")
                            nc.tensor.matmul(
                                ps2[:], hselQ_t[:, 128 * h:128 * h + 128],
                                sq[:], start=True, stop=True)
                            nc.vector.tensor_copy(
                                qhT[h][a:a + 1, 512 * sj:512 * sj + 512],
                                ps2[a:a + 1, :])

                preamble(0)

                # vp projection (raw v, no scaling needed)
                for w in range(NST):
                    ps = PS.tile([128, 512], F32, name="psv", tag="ps")
                    for k in range(NKT):
                        nc.tensor.matmul(
                            ps[:, 0:HD], qT[k][:, 128 * w:128 * w + 128],
                            wvb[k][:], start=(k == 0), stop=(k == NKT - 1))
                    nc.scalar.copy(vp[w][:], ps[:, 0:HD])

                # attention: per s-chunk, qk(+aug) -> exp(+kn bias) -> sv
                for sj in range(NSC):
                    ot_ps = [OT.tile([128, 512], F32, name="ot", tag="ot")
                             for m in range(2)]
                    for m in range(2):
                        nc.vector.memset(ot_ps[m][:], 0.0)
                    # 2-deep software pipeline: sv for tile-pair j runs
                    # while qk/exp for pairs j+1, j+2 fill the PE/Act queues,
                    # so the PE never waits on the exp chain.
                    tis = list(range(4 * sj, NST))
                    DEPTH = 2
                    ets = {}
                    for idx in range(len(tis) + DEPTH):
                        if idx < len(tis):
                            ti = tis[idx]
                            r = idx
                            span = min(512, 128 * (r + 1))
                            diag = r < 4
                            for m in range(2):
                                for hl in range(2):
                                    h = 2 * m + hl
                                    lo, hi = HEAD_SLICE[h]
                                    qk2 = PS.tile([128, 512], F32, name="qk",
                                                  tag="ps")
                                    nc.tensor.matmul(
                                        qk2[:, 0:span],
                                        khT[h][lo:hi,
                                               128 * ti:128 * ti + 128],
                                        qhT[h][lo:hi,
                                               512 * sj:512 * sj + span],
                                        start=True, stop=True)
                                    et2 = EP.tile([128, 512], BF16,
                                                  name="et", tag="et")
                                    nc.scalar.activation(
                                        et2[:, 0:span], qk2[:, 0:span],
                                        AF.Exp,
                                        bias=knT[:, 4 * ti + h:
                                                 4 * ti + h + 1])
                                    if diag:
                                        band = et2[:, 128 * r:128 * r + 128]
                                        nc.vector.tensor_tensor(
                                            band, band, mask_t[:],
                                            op=ALU.mult)
                                    ets[(idx, m, hl)] = et2
                            if r == 2 and sj < NSC - 1:
                                preamble(sj + 1)
                        j = idx - DEPTH
                        if j >= 0:
                            ti2 = tis[j]
                            sp2 = min(512, 128 * (j + 1))
                            last = (ti2 == NST - 1)
                            for m in range(2):
                                for hl in range(2):
                                    h = 2 * m + hl
                                    base = 64 * hl
                                    et2 = ets.pop((j, m, hl))
                                    nc.tensor.matmul(
                                        ot_ps[m][base:base + 64, 0:sp2],
                                        vp[ti2][:, 64 * h:64 * h + 64],
                                        et2[:, 0:sp2],
                                        start=False, stop=last,
                                        skip_group_check=True)
                    for m in range(2):
                        nc.vector.tensor_copy(
                            outT[m][:, 512 * sj:512 * sj + 512], ot_ps[m][:])

                    # final partial for this chunk's s-windows (bf16 out)
                    for w in range(4 * sj, 4 * sj + 4):
                        fo = EP.tile([128, E], BF16, name="fo", tag="fo")
                        for n in range(2):
                            ps = PS.tile([128, 512], F32, name="fp",
                                         tag="ps")
                            for k in range(2):
                                nc.tensor.matmul(
                                    ps[:, 0:512],
                                    outT[k][:, 128 * w:128 * w + 128],
                                    wob[k][:, 512 * n:512 * n + 512],
                                    start=(k == 0), stop=(k == 1))
                            nc.vector.tensor_copy(
                                fo[:, 512 * n:512 * n + 512], ps[:, 0:512])
                        nc.sync.dma_start(out_d[128 * w:128 * w + 128, :],
                                          fo[:])

    nc.compile()
    _nc_cache['nc'] = nc
    return nc


def shard_inputs(q, Wq, Wk, Wv, Wo, gamma):
    in_maps = []
    for c in range(N_CORES):
        b, g = c // 4, c % 4
        cols = slice(HD * g, HD * (g + 1))
        gam = gamma[HPC * g:HPC * (g + 1)].astype(np.float64)
        c_h = gam * SCALE
        wk_scaled = Wk[:, cols].astype(np.float64).copy()
        for h in range(HPC):
            wk_scaled[:, 64 * h:64 * h + 64] *= 2.0 * c_h[h]
        hselK = np.zeros((128, HPC), dtype=np.float64)
        hselQ = np.zeros((128, 128 * HPC), dtype=np.float64)
        for i in range(HPC):
            lo, hi = HEAD_DATA[i]
            hselK[lo:hi, i] = -1.0 / (4.0 * c_h[i])
            hselQ[lo:hi, 128 * i:128 * i + 128] = -c_h[i]
        in_maps.append(dict(
            qbT=np.ascontiguousarray(q[b].T.astype(ml_dtypes.bfloat16)),
            wq=np.ascontiguousarray(Wq[:, cols].astype(ml_dtypes.bfloat16)),
            wk=np.ascontiguousarray(
                wk_scaled.astype(np.float32).astype(ml_dtypes.bfloat16)),
            wv=np.ascontiguousarray(Wv[:, cols].astype(ml_dtypes.bfloat16)),
            wo=np.ascontiguousarray(Wo[cols, :].astype(ml_dtypes.bfloat16)),
            hselK=np.ascontiguousarray(hselK.astype(ml_dtypes.bfloat16)),
            hselQ128=np.ascontiguousarray(hselQ.astype(ml_dtypes.bfloat16)),
        ))
    return in_maps


def kernel(q, Wq, Wk, Wv, Wo, gamma):
    q = np.asarray(q, dtype=np.float32)
    Wq = np.asarray(Wq, dtype=np.float32)
    Wk = np.asarray(Wk, dtype=np.float32)
    Wv = np.asarray(Wv, dtype=np.float32)
    Wo = np.asarray(Wo, dtype=np.float32)
    gamma = np.asarray(gamma, dtype=np.float32)

    nc = build_graph()
    in_maps = shard_inputs(q, Wq, Wk, Wv, Wo, gamma)
    res = bass_utils.run_bass_kernel_spmd(nc, in_maps,
                                          core_ids=list(range(N_CORES)))
    out = np.zeros((B, S, E), dtype=np.float32)
    for c in range(N_CORES):
        out[c // 4] += np.asarray(res.results[c]["out"], dtype=np.float32)
    return out
